# revision 21
# baseline (speedup 1.0000x reference)
"""Trainium2 Bass kernel for CrossModalAttention (B=65536, DIM=768, 3 heads,
q_len=1, kv_len=3) — data-parallel over 8 NeuronCores.

Layout "T": all on-chip activations are [dim, rows]; projections contract
dim-chunks on the partition axis with zero on-chip transposes.

v3 over the original baseline:
  * mix-V: since softmax weights sum to 1,
      ctx_h = Wv_h (sum_s attn_{s,h} k_s) + bv_h,
    so the three per-key V projections (108 matmuls/tile) collapse into one
    projection of the attn-mixed raw keys (36 matmuls/tile); the mixing is
    cheap DVE/GpSimd elementwise work.
  * bf16 staging of q/k and all four weight matrices: full PE rate, half
    the DMA and SBUF, letting k triple-buffer across the software pipeline.
  * LN rstd = ACT Sqrt(DVE reciprocal(var)) instead of a Newton chain;
    gamma/beta applied by ACT with per-partition scale+bias.
  * full unroll (no tc.For_i): no all-engine barrier per tile, and tile
    i's softmax/mix/output phase is emitted interleaved with tile i+1's
    projection phase so the PE never drains on the ACT/DVE serial chains.
"""

import sys

if "/opt/trn_rl_repo" not in sys.path:
    sys.path.insert(0, "/opt/trn_rl_repo")

from contextlib import ExitStack

import numpy as np
import ml_dtypes

import concourse.bass as bass
import concourse.bacc as bacc
import concourse.mybir as mybir
import concourse.tile as tile
from concourse.bass_utils import run_bass_kernel_spmd

DIM = 768
P = 128
KO = DIM // P  # 6 chunks of the feature dim
H = 3          # heads
S = 3          # kv positions
SH = S * H
NCORES = 8
B = 65536
BCORE = B // NCORES
R = 512        # rows (batch elements) per tile iteration
EPS = 1e-5

AF = mybir.ActivationFunctionType
OP = mybir.AluOpType
F32 = mybir.dt.float32
F32R = mybir.dt.float32r
BF16 = mybir.dt.bfloat16

BF16NP = ml_dtypes.bfloat16


def _mm(nc, out, lhsT, rhs, start, stop):
    nc.tensor.matmul(out, lhsT=lhsT, rhs=rhs, start=start, stop=stop)


def build_nc(b_core=BCORE, r=R, niter_cap=None):
    assert b_core % r == 0
    niter = b_core // r
    if niter_cap is not None:
        niter = min(niter, niter_cap)
    nc = bacc.Bacc()

    qT = nc.dram_tensor("qT", [DIM, b_core], BF16, kind="ExternalInput")
    kT = [
        nc.dram_tensor(f"k{s}T", [DIM, b_core], BF16, kind="ExternalInput")
        for s in range(S)
    ]
    wT_d = {
        n: nc.dram_tensor(n, [DIM, DIM], BF16, kind="ExternalInput")
        for n in ("wqT", "wkT", "wvT", "woT")
    }
    bias_d = {
        n: nc.dram_tensor(n, [DIM], F32, kind="ExternalInput")
        for n in ("bq", "bk", "bv", "bo")
    }
    gamma_d = nc.dram_tensor("gamma", [DIM], F32, kind="ExternalInput")
    beta_d = nc.dram_tensor("beta", [DIM], F32, kind="ExternalInput")
    selscore_d = nc.dram_tensor("selscore", [P, SH, SH], BF16, kind="ExternalInput")
    sel3_d = nc.dram_tensor("sel3", [SH, H], F32R, kind="ExternalInput")
    sel9_d = nc.dram_tensor("sel9", [H, SH], F32R, kind="ExternalInput")
    rowsel9_d = nc.dram_tensor("rowsel9", [SH, SH, P], BF16, kind="ExternalInput")
    ones128_d = nc.dram_tensor("ones128", [P, 1], F32R, kind="ExternalInput")
    ones1_d = nc.dram_tensor("ones1", [1, P], F32R, kind="ExternalInput")
    out_d = nc.dram_tensor("out", [DIM, b_core], F32, kind="ExternalOutput")

    qT_t = qT[:].rearrange("(ko p) n -> p ko n", p=P)
    kT_t = [k[:].rearrange("(ko p) n -> p ko n", p=P) for k in kT]
    out_t = out_d[:].rearrange("(ko p) n -> p ko n", p=P)

    with tile.TileContext(nc) as tc, ExitStack() as ctx:
        ctx.enter_context(nc.allow_low_precision(reason="bf16/fp32r matmul pipeline"))
        wpool = ctx.enter_context(tc.tile_pool(name="wpool", bufs=1))
        qpool = ctx.enter_context(tc.tile_pool(name="qpool", bufs=3))
        kpool = ctx.enter_context(tc.tile_pool(name="kpool", bufs=9))
        qcpool = ctx.enter_context(tc.tile_pool(name="qcpool", bufs=2))
        kcpool = ctx.enter_context(tc.tile_pool(name="kcpool", bufs=3))
        prodpool = ctx.enter_context(tc.tile_pool(name="prodpool", bufs=7))
        mpool = ctx.enter_context(tc.tile_pool(name="mpool", bufs=3))
        ctxpool = ctx.enter_context(tc.tile_pool(name="ctxpool", bufs=1))
        xpool = ctx.enter_context(tc.tile_pool(name="xpool", bufs=1))
        xsqpool = ctx.enter_context(tc.tile_pool(name="xsqpool", bufs=1))
        tmppool = ctx.enter_context(tc.tile_pool(name="tmppool", bufs=3))
        t1pool = ctx.enter_context(tc.tile_pool(name="t1pool", bufs=3))
        abpool = ctx.enter_context(tc.tile_pool(name="abpool", bufs=6))
        ypool = ctx.enter_context(tc.tile_pool(name="ypool", bufs=2))
        smpool = ctx.enter_context(tc.tile_pool(name="smpool", bufs=1))
        psq = ctx.enter_context(tc.tile_pool(name="psq", bufs=3, space="PSUM"))
        pssc = ctx.enter_context(tc.tile_pool(name="pssc", bufs=2, space="PSUM"))
        psbc = ctx.enter_context(tc.tile_pool(name="psbc", bufs=3, space="PSUM"))

        # ---- resident constants ----
        w_sb = {}
        for n in wT_d:
            w = wpool.tile([P, KO, KO, P], BF16, name=f"w_{n}")
            nc.sync.dma_start(
                out=w,
                in_=wT_d[n][:].rearrange("(ko p) (jo m) -> p ko jo m", p=P, m=P),
            )
            w_sb[n] = w
        bias_sb = {}
        for n in bias_d:
            t = wpool.tile([P, KO], F32, name=f"b_{n}")
            nc.sync.dma_start(out=t, in_=bias_d[n][:].rearrange("(jo m) -> m jo", m=P))
            bias_sb[n] = t
        beta_col = wpool.tile([P, KO], F32, name="beta_col")
        nc.sync.dma_start(out=beta_col, in_=beta_d[:].rearrange("(jo m) -> m jo", m=P))
        gamma_col = wpool.tile([P, KO], F32, name="gamma_col")
        nc.sync.dma_start(out=gamma_col, in_=gamma_d[:].rearrange("(jo m) -> m jo", m=P))
        ones1 = wpool.tile([1, P], F32R, name="ones1")
        nc.sync.dma_start(out=ones1, in_=ones1_d[:])
        selscore_sb = wpool.tile([P, SH, SH], BF16, name="selscore_sb")
        nc.sync.dma_start(out=selscore_sb, in_=selscore_d[:])
        sel3_sb = wpool.tile([SH, H], F32R, name="sel3_sb")
        nc.sync.dma_start(out=sel3_sb, in_=sel3_d[:])
        sel9_sb = wpool.tile([H, SH], F32R, name="sel9_sb")
        nc.sync.dma_start(out=sel9_sb, in_=sel9_d[:])
        rowsel9_sb = wpool.tile([SH, SH, P], BF16, name="rowsel9_sb")
        nc.sync.dma_start(out=rowsel9_sb, in_=rowsel9_d[:])
        ones128 = wpool.tile([P, 1], F32R, name="ones128")
        nc.sync.dma_start(out=ones128, in_=ones128_d[:])

        state = {}

        def load_tile(j):
            q_in = qpool.tile([P, KO, r], BF16, name="q_in", tag="q_in")
            nc.sync.dma_start(out=q_in, in_=qT_t[:, :, bass.ds(j * r, r)])
            k_in = []
            for s in range(S):
                kt = kpool.tile([P, KO, r], BF16, name=f"k_in{s}", tag="k_in")
                nc.sync.dma_start(out=kt, in_=kT_t[s][:, :, bass.ds(j * r, r)])
                k_in.append(kt)
            state[("q", j)] = q_in
            state[("k", j)] = k_in

        def emit_step(i):
            """Interleaves phase-2 of tile i-1 with phase-1 of tile i."""
            cur = i < niter
            prev = i > 0
            ip = i - 1

            # prefetch tile i+1 inputs
            if i + 1 < niter:
                load_tile(i + 1)

            # ---- softmax for tile i-1 (exp was done last step) ----
            if prev:
                e = state.pop(("exp", ip))
                den_ps = pssc.tile([H, r], F32, name="den_ps", tag="sc")
                _mm(nc, den_ps, sel3_sb[:], e[:], True, True)
                rden = smpool.tile([H, r], F32R, name="rden", tag="rden")
                nc.vector.reciprocal(out=rden, in_=den_ps)
                denb_ps = pssc.tile([SH, r], F32, name="denb_ps", tag="sc")
                _mm(nc, denb_ps, sel9_sb[:], rden[:], True, True)
                attn = smpool.tile([SH, r], BF16, name="attn", tag="attn")
                nc.vector.tensor_mul(out=attn, in0=e, in1=denb_ps)
                kp_in = state.pop(("k", ip))
                m_tiles = []

            # ---- tile i: Q projection ----
            if cur:
                q_in = state[("q", i)]
                k_in = state[("k", i)]
                qc = qcpool.tile([P, KO, r], BF16, name="qc", tag="qc")
                for jo in range(KO):
                    qp = psq.tile([P, r], F32, name="qp", tag="mm")
                    for ko in range(KO):
                        _mm(nc, qp, w_sb["wqT"][:, ko, jo, :], q_in[:, ko, :],
                            ko == 0, ko == KO - 1)
                    nc.scalar.activation(
                        out=qc[:, jo, :], in_=qp, func=AF.Identity,
                        bias=bias_sb["bq"][:, jo : jo + 1],
                    )
                scores_ps = pssc.tile([SH, r], F32, name="scores_ps", tag="sc")
                prods = {}

            def emit_ab_mix(h):
                """attn-row broadcasts for head h (PE into PSUM, ACT-copied to
                bf16 SBUF) + raw-key mixing in pure-bf16 DVE (4x mode)."""
                abs_ = []
                for s in range(S):
                    ab = psbc.tile([P, r], F32, name="ab", tag="bc")
                    _mm(nc, ab, rowsel9_sb[:, s * H + h, :], attn[:], True, True)
                    ab_sb = abpool.tile([P, r], BF16, name="ab_sb", tag="ab")
                    nc.scalar.copy(out=ab_sb, in_=ab)
                    abs_.append(ab_sb)
                m_h = mpool.tile([P, KO, r], BF16, name="m_h", tag="m")
                for ko in range(KO):
                    nc.vector.tensor_mul(
                        out=m_h[:, ko, :], in0=abs_[0], in1=kp_in[0][:, ko, :]
                    )
                    for s in (1, 2):
                        t = tmppool.tile([P, r], BF16, name="t", tag="tmp")
                        nc.vector.tensor_mul(out=t, in0=abs_[s], in1=kp_in[s][:, ko, :])
                        nc.gpsimd.tensor_add(
                            out=m_h[:, ko, :], in0=m_h[:, ko, :], in1=t
                        )
                m_tiles.append(m_h)

            def emit_kproj(s):
                """K projection + score products for kv position s (tile i)."""
                for jo in range(KO):
                    kp = psq.tile([P, r], F32, name="kp", tag="mm")
                    for ko in range(KO):
                        _mm(nc, kp, w_sb["wkT"][:, ko, jo, :], k_in[s][:, ko, :],
                            ko == 0, ko == KO - 1)
                    kc = kcpool.tile([P, r], BF16, name="kc", tag="kc")
                    nc.scalar.activation(
                        out=kc, in_=kp, func=AF.Identity,
                        bias=bias_sb["bk"][:, jo : jo + 1],
                    )
                    pr = prodpool.tile([P, r], BF16, name="pr", tag="pr")
                    nc.vector.tensor_mul(out=pr, in0=qc[:, jo, :], in1=kc)
                    prods[(s, jo)] = pr

            def emit_sel(s, last_s):
                for jo in range(KO):
                    _mm(nc, scores_ps, selscore_sb[:, s * H + jo // 2, :],
                        prods.pop((s, jo)),
                        s == 0 and jo == 0, last_s and jo == KO - 1)

            # interleave: ab/mix groups (tile i-1) between K-proj blocks
            # (tile i) so the PE never waits on the DVE/GpSimd mixing, and
            # sel reductions trail their K block so the products are ready.
            if prev and cur:
                emit_kproj(0)
                emit_ab_mix(0)
                emit_sel(0, False)
                emit_kproj(1)
                emit_ab_mix(1)
                emit_sel(1, False)
                emit_kproj(2)
                emit_ab_mix(2)
                emit_sel(2, True)
            elif cur:
                for s in range(S):
                    emit_kproj(s)
                for s in range(S):
                    emit_sel(s, s == S - 1)
            elif prev:
                for h in range(H):
                    emit_ab_mix(h)

            if cur:
                exp_sb = smpool.tile([SH, r], F32R, name="exp_sb", tag="exp")
                nc.scalar.activation(
                    out=exp_sb, in_=scores_ps, func=AF.Exp, scale=1.0 / 16.0
                )
                state[("exp", i)] = exp_sb

            # ---- tile i-1: V projection of mixed keys, O projection,
            # residual + LayerNorm, store ----
            if prev:
                ctx_sb = ctxpool.tile([P, KO, r], BF16, name="ctx_sb", tag="ctx")
                for h in range(H):
                    for b in range(2):
                        jo = 2 * h + b
                        vp = psq.tile([P, r], F32, name="vp", tag="mm")
                        for ko in range(KO):
                            _mm(nc, vp, w_sb["wvT"][:, ko, jo, :],
                                m_tiles[h][:, ko, :], ko == 0, ko == KO - 1)
                        nc.scalar.activation(
                            out=ctx_sb[:, jo, :], in_=vp, func=AF.Identity,
                            bias=bias_sb["bv"][:, jo : jo + 1],
                        )

                qp_in = state.pop(("q", ip))
                x_sb = xpool.tile([P, KO, r], F32R, name="x_sb", tag="x")
                sx_ps = pssc.tile([1, r], F32, name="sx_ps", tag="sc")
                sxx_ps = pssc.tile([1, r], F32, name="sxx_ps", tag="sc")
                xsqs = []

                def emit_sum(jo):
                    _mm(nc, sx_ps, ones128[:], x_sb[:, jo, :], jo == 0, jo == KO - 1)
                    _mm(nc, sxx_ps, ones128[:], xsqs[jo][:], jo == 0, jo == KO - 1)

                for jo in range(KO):
                    op_ps = psq.tile([P, r], F32, name="op_ps", tag="mm")
                    for ko in range(KO):
                        _mm(nc, op_ps, w_sb["woT"][:, ko, jo, :], ctx_sb[:, ko, :],
                            ko == 0, ko == KO - 1)
                    nc.vector.scalar_tensor_tensor(
                        out=x_sb[:, jo, :], in0=op_ps,
                        scalar=bias_sb["bo"][:, jo : jo + 1],
                        in1=qp_in[:, jo, :], op0=OP.add, op1=OP.add,
                    )
                    xsq = xsqpool.tile([P, r], F32R, name="xsq", tag="xsq")
                    nc.scalar.activation(out=xsq, in_=x_sb[:, jo, :], func=AF.Square)
                    xsqs.append(xsq)
                    if jo >= 1:
                        emit_sum(jo - 1)
                emit_sum(KO - 1)

                mv0 = smpool.tile([1, r], F32, name="mv0", tag="mv0")
                nc.vector.tensor_scalar_mul(out=mv0, in0=sx_ps, scalar1=1.0 / DIM)
                mv1 = smpool.tile([1, r], F32, name="mv1", tag="mv1")
                nc.vector.tensor_scalar_mul(out=mv1, in0=sxx_ps, scalar1=1.0 / DIM)
                mu2 = smpool.tile([1, r], F32, name="mu2", tag="mu2")
                nc.vector.tensor_mul(out=mu2, in0=mv0, in1=mv0)
                # mv1 <- var + eps
                nc.vector.scalar_tensor_tensor(
                    out=mv1, in0=mv1, scalar=EPS, in1=mu2, op0=OP.add, op1=OP.subtract
                )
                rvar = smpool.tile([1, r], F32, name="rvar", tag="rvar")
                nc.vector.reciprocal(out=rvar, in_=mv1)
                rstd = smpool.tile([1, r], F32R, name="rstd", tag="rstd")
                nc.scalar.activation(out=rstd, in_=rvar, func=AF.Sqrt)
                m2 = smpool.tile([1, r], F32R, name="m2", tag="m2")
                nc.vector.tensor_mul(out=m2, in0=mv0, in1=rstd)

                rstd_b = psbc.tile([P, r], F32, name="rstd_b", tag="bc")
                _mm(nc, rstd_b, ones1[:], rstd[:], True, True)
                m2_b = psbc.tile([P, r], F32, name="m2_b", tag="bc")
                _mm(nc, m2_b, ones1[:], m2[:], True, True)
                for jo in range(KO):
                    t1 = t1pool.tile([P, r], F32R, name="t1", tag="t1")
                    nc.vector.tensor_mul(out=t1, in0=x_sb[:, jo, :], in1=rstd_b)
                    nc.vector.tensor_sub(out=t1, in0=t1, in1=m2_b)
                    y = ypool.tile([P, r], F32, name="y", tag="y")
                    nc.scalar.activation(
                        out=y, in_=t1, func=AF.Identity,
                        bias=beta_col[:, jo : jo + 1],
                        scale=gamma_col[:, jo : jo + 1],
                    )
                    nc.gpsimd.dma_start(out=out_t[:, jo, bass.ds(ip * r, r)], in_=y)

        load_tile(0)
        for i in range(niter + 1):
            emit_step(i)

    nc.compile()
    return nc


def make_consts():
    selscore = np.broadcast_to(np.eye(SH, dtype=BF16NP)[None], (P, SH, SH))
    selscore = np.ascontiguousarray(selscore)
    rowsel9 = np.ascontiguousarray(
        np.broadcast_to(np.eye(SH, dtype=BF16NP)[:, :, None], (SH, SH, P))
    )
    k = np.arange(SH)
    sel3 = (k[:, None] % H == np.arange(H)[None, :]).astype(np.float32)
    sel9 = (k[None, :] % H == np.arange(H)[:, None]).astype(np.float32)
    return selscore, rowsel9, sel3, sel9


def make_in_maps(inputs, b_core=BCORE, ncores=NCORES):
    f = np.float32
    q = np.asarray(inputs["query"], f)
    keys = [np.asarray(inputs[f"key{s}"], f) for s in range(S)]
    shared = {
        "wqT": np.ascontiguousarray(np.asarray(inputs["Wq"], f).T).astype(BF16NP),
        "wkT": np.ascontiguousarray(np.asarray(inputs["Wk"], f).T).astype(BF16NP),
        "wvT": np.ascontiguousarray(np.asarray(inputs["Wv"], f).T).astype(BF16NP),
        "woT": np.ascontiguousarray(np.asarray(inputs["Wo"], f).T).astype(BF16NP),
        "bq": np.asarray(inputs["bq"], f),
        "bk": np.asarray(inputs["bk"], f),
        "bv": np.asarray(inputs["bv"], f),
        "bo": np.asarray(inputs["bo"], f),
        "gamma": np.asarray(inputs["gamma"], f),
        "beta": np.asarray(inputs["beta"], f),
    }
    selscore, rowsel9, sel3, sel9 = make_consts()
    shared.update({"selscore": selscore, "rowsel9": rowsel9, "sel3": sel3,
                   "sel9": sel9,
                   "ones128": np.ones((P, 1), f), "ones1": np.ones((1, P), f)})
    in_maps = []
    for c in range(ncores):
        sl = slice(c * b_core, (c + 1) * b_core)
        m = dict(shared)
        m["qT"] = np.ascontiguousarray(q[sl].T).astype(BF16NP)
        for s in range(S):
            m[f"k{s}T"] = np.ascontiguousarray(keys[s][sl].T).astype(BF16NP)
        in_maps.append(m)
    return in_maps


_NC_CACHE = {}


def _get_nc(b_core=BCORE, r=R, niter_cap=None):
    key = (b_core, r, niter_cap)
    if key not in _NC_CACHE:
        _NC_CACHE[key] = build_nc(b_core, r, niter_cap)
    return _NC_CACHE[key]


def run(inputs, trace=False, tmpdir=None):
    """Run on 8 NeuronCores; returns (full output, BassKernelResults)."""
    nc = _get_nc()
    in_maps = make_in_maps(inputs)
    res = run_bass_kernel_spmd(
        nc, in_maps, core_ids=list(range(NCORES)), trace=trace, tmpdir=tmpdir
    )
    y = np.empty((B, DIM), np.float32)
    for c in range(NCORES):
        y[c * BCORE : (c + 1) * BCORE] = res.results[c]["out"].T
    return y, res


def kernel(**inputs):
    y, _ = run(inputs)
    return y


# revision 22
# speedup vs baseline: 1.3478x; 1.3478x over previous
"""Trainium2 Bass kernel for CrossModalAttention (B=65536, DIM=768, 3 heads,
q_len=1, kv_len=3) — data-parallel over 8 NeuronCores.

Layout "T": all on-chip activations are [dim, rows]; projections contract
dim-chunks on the partition axis with zero on-chip transposes.

v3 over the original baseline:
  * mix-V: since softmax weights sum to 1,
      ctx_h = Wv_h (sum_s attn_{s,h} k_s) + bv_h,
    so the three per-key V projections (108 matmuls/tile) collapse into one
    projection of the attn-mixed raw keys (36 matmuls/tile); the mixing is
    cheap DVE/GpSimd elementwise work.
  * bf16 staging of q/k and all four weight matrices: full PE rate, half
    the DMA and SBUF, letting k triple-buffer across the software pipeline.
  * LN rstd = ACT Sqrt(DVE reciprocal(var)) instead of a Newton chain;
    gamma/beta applied by ACT with per-partition scale+bias.
  * full unroll (no tc.For_i): no all-engine barrier per tile, and tile
    i's softmax/mix/output phase is emitted interleaved with tile i+1's
    projection phase so the PE never drains on the ACT/DVE serial chains.
"""

import sys

if "/opt/trn_rl_repo" not in sys.path:
    sys.path.insert(0, "/opt/trn_rl_repo")

from contextlib import ExitStack

import numpy as np
import ml_dtypes

import concourse.bass as bass
import concourse.bacc as bacc
import concourse.mybir as mybir
import concourse.tile as tile
from concourse.bass_utils import run_bass_kernel_spmd

DIM = 768
P = 128
KO = DIM // P  # 6 chunks of the feature dim
H = 3          # heads
S = 3          # kv positions
SH = S * H
NCORES = 8
B = 65536
BCORE = B // NCORES
R = 512        # rows (batch elements) per tile iteration
EPS = 1e-5

AF = mybir.ActivationFunctionType
OP = mybir.AluOpType
F32 = mybir.dt.float32
F32R = mybir.dt.float32r
BF16 = mybir.dt.bfloat16

BF16NP = ml_dtypes.bfloat16


def _mm(nc, out, lhsT, rhs, start, stop):
    nc.tensor.matmul(out, lhsT=lhsT, rhs=rhs, start=start, stop=stop)


def build_nc(b_core=BCORE, r=R, niter_cap=None):
    assert b_core % r == 0
    niter = b_core // r
    if niter_cap is not None:
        niter = min(niter, niter_cap)
    nc = bacc.Bacc()

    qT = nc.dram_tensor("qT", [DIM, b_core], BF16, kind="ExternalInput")
    kT = [
        nc.dram_tensor(f"k{s}T", [DIM, b_core], BF16, kind="ExternalInput")
        for s in range(S)
    ]
    wT_d = {
        n: nc.dram_tensor(n, [DIM, DIM], BF16, kind="ExternalInput")
        for n in ("wqT", "wkT", "wvT", "woT")
    }
    bias_d = {
        n: nc.dram_tensor(n, [DIM], F32, kind="ExternalInput")
        for n in ("bq", "bk", "bv", "bo")
    }
    gamma_d = nc.dram_tensor("gamma", [DIM], F32, kind="ExternalInput")
    beta_d = nc.dram_tensor("beta", [DIM], F32, kind="ExternalInput")
    selscore_d = nc.dram_tensor("selscore", [P, SH, SH], BF16, kind="ExternalInput")
    sel3_d = nc.dram_tensor("sel3", [SH, H], F32R, kind="ExternalInput")
    sel9_d = nc.dram_tensor("sel9", [H, SH], F32R, kind="ExternalInput")
    rowsel9_d = nc.dram_tensor("rowsel9", [SH, SH, P], BF16, kind="ExternalInput")
    ones128_d = nc.dram_tensor("ones128", [P, 1], F32R, kind="ExternalInput")
    ones1_d = nc.dram_tensor("ones1", [1, P], F32R, kind="ExternalInput")
    out_d = nc.dram_tensor("out", [DIM, b_core], F32, kind="ExternalOutput")

    qT_t = qT[:].rearrange("(ko p) n -> p ko n", p=P)
    kT_t = [k[:].rearrange("(ko p) n -> p ko n", p=P) for k in kT]
    out_t = out_d[:].rearrange("(ko p) n -> p ko n", p=P)

    with tile.TileContext(nc) as tc, ExitStack() as ctx:
        ctx.enter_context(nc.allow_low_precision(reason="bf16/fp32r matmul pipeline"))
        wpool = ctx.enter_context(tc.tile_pool(name="wpool", bufs=1))
        qpool = ctx.enter_context(tc.tile_pool(name="qpool", bufs=3))
        kpool = ctx.enter_context(tc.tile_pool(name="kpool", bufs=9))
        qcpool = ctx.enter_context(tc.tile_pool(name="qcpool", bufs=2))
        kcpool = ctx.enter_context(tc.tile_pool(name="kcpool", bufs=3))
        prodpool = ctx.enter_context(tc.tile_pool(name="prodpool", bufs=7))
        mpool = ctx.enter_context(tc.tile_pool(name="mpool", bufs=3))
        ctxpool = ctx.enter_context(tc.tile_pool(name="ctxpool", bufs=1))
        xpool = ctx.enter_context(tc.tile_pool(name="xpool", bufs=1))
        xsqpool = ctx.enter_context(tc.tile_pool(name="xsqpool", bufs=1))
        tmppool = ctx.enter_context(tc.tile_pool(name="tmppool", bufs=3))
        t1pool = ctx.enter_context(tc.tile_pool(name="t1pool", bufs=3))
        abpool = ctx.enter_context(tc.tile_pool(name="abpool", bufs=6))
        ypool = ctx.enter_context(tc.tile_pool(name="ypool", bufs=2))
        smpool = ctx.enter_context(tc.tile_pool(name="smpool", bufs=1))
        psq = ctx.enter_context(tc.tile_pool(name="psq", bufs=3, space="PSUM"))
        pssc = ctx.enter_context(tc.tile_pool(name="pssc", bufs=2, space="PSUM"))
        psbc = ctx.enter_context(tc.tile_pool(name="psbc", bufs=3, space="PSUM"))

        # ---- resident constants ----
        w_sb = {}
        for n in wT_d:
            w = wpool.tile([P, KO, KO, P], BF16, name=f"w_{n}")
            nc.sync.dma_start(
                out=w,
                in_=wT_d[n][:].rearrange("(ko p) (jo m) -> p ko jo m", p=P, m=P),
            )
            w_sb[n] = w
        bias_sb = {}
        for n in bias_d:
            t = wpool.tile([P, KO], F32, name=f"b_{n}")
            nc.sync.dma_start(out=t, in_=bias_d[n][:].rearrange("(jo m) -> m jo", m=P))
            bias_sb[n] = t
        beta_col = wpool.tile([P, KO], F32, name="beta_col")
        nc.sync.dma_start(out=beta_col, in_=beta_d[:].rearrange("(jo m) -> m jo", m=P))
        gamma_col = wpool.tile([P, KO], F32, name="gamma_col")
        nc.sync.dma_start(out=gamma_col, in_=gamma_d[:].rearrange("(jo m) -> m jo", m=P))
        ones1 = wpool.tile([1, P], F32R, name="ones1")
        nc.sync.dma_start(out=ones1, in_=ones1_d[:])
        selscore_sb = wpool.tile([P, SH, SH], BF16, name="selscore_sb")
        nc.sync.dma_start(out=selscore_sb, in_=selscore_d[:])
        sel3_sb = wpool.tile([SH, H], F32R, name="sel3_sb")
        nc.sync.dma_start(out=sel3_sb, in_=sel3_d[:])
        sel9_sb = wpool.tile([H, SH], F32R, name="sel9_sb")
        nc.sync.dma_start(out=sel9_sb, in_=sel9_d[:])
        rowsel9_sb = wpool.tile([SH, SH, P], BF16, name="rowsel9_sb")
        nc.sync.dma_start(out=rowsel9_sb, in_=rowsel9_d[:])
        ones128 = wpool.tile([P, 1], F32R, name="ones128")
        nc.sync.dma_start(out=ones128, in_=ones128_d[:])

        state = {}

        def load_tile(j):
            q_in = qpool.tile([P, KO, r], BF16, name="q_in", tag="q_in")
            nc.sync.dma_start(out=q_in, in_=qT_t[:, :, bass.ds(j * r, r)])
            k_in = []
            for s in range(S):
                kt = kpool.tile([P, KO, r], BF16, name=f"k_in{s}", tag="k_in")
                nc.sync.dma_start(out=kt, in_=kT_t[s][:, :, bass.ds(j * r, r)])
                k_in.append(kt)
            state[("q", j)] = q_in
            state[("k", j)] = k_in

        def emit_step(i):
            """Interleaves phase-2 of tile i-1 with phase-1 of tile i."""
            cur = i < niter
            prev = i > 0
            ip = i - 1

            # prefetch tile i+1 inputs
            if i + 1 < niter:
                load_tile(i + 1)

            # ---- softmax for tile i-1 (exp was done last step) ----
            if prev:
                e = state.pop(("exp", ip))
                den_ps = pssc.tile([H, r], F32, name="den_ps", tag="sc")
                _mm(nc, den_ps, sel3_sb[:], e[:], True, True)
                rden = smpool.tile([H, r], F32R, name="rden", tag="rden")
                nc.vector.reciprocal(out=rden, in_=den_ps)
                denb_ps = pssc.tile([SH, r], F32, name="denb_ps", tag="sc")
                _mm(nc, denb_ps, sel9_sb[:], rden[:], True, True)
                attn = smpool.tile([SH, r], BF16, name="attn", tag="attn")
                nc.vector.tensor_mul(out=attn, in0=e, in1=denb_ps)
                kp_in = state.pop(("k", ip))
                m_tiles = []

            # ---- tile i: Q projection ----
            if cur:
                q_in = state[("q", i)]
                k_in = state[("k", i)]
                qc = qcpool.tile([P, KO, r], BF16, name="qc", tag="qc")
                for jo in range(KO):
                    qp = psq.tile([P, r], F32, name="qp", tag="mm")
                    for ko in range(KO):
                        _mm(nc, qp, w_sb["wqT"][:, ko, jo, :], q_in[:, ko, :],
                            ko == 0, ko == KO - 1)
                    nc.scalar.activation(
                        out=qc[:, jo, :], in_=qp, func=AF.Identity,
                        bias=bias_sb["bq"][:, jo : jo + 1],
                    )
                scores_ps = pssc.tile([SH, r], F32, name="scores_ps", tag="sc")
                prods = {}

            def emit_ab_mix(h):
                """attn-row broadcasts for head h (PE into PSUM, ACT-copied to
                bf16 SBUF) + raw-key mixing in pure-bf16 DVE (4x mode)."""
                abs_ = []
                for s in range(S):
                    ab = psbc.tile([P, r], F32, name="ab", tag="bc")
                    _mm(nc, ab, rowsel9_sb[:, s * H + h, :], attn[:], True, True)
                    ab_sb = abpool.tile([P, r], BF16, name="ab_sb", tag="ab")
                    nc.scalar.copy(out=ab_sb, in_=ab)
                    abs_.append(ab_sb)
                m_h = mpool.tile([P, KO, r], BF16, name="m_h", tag="m")
                for ko in range(KO):
                    nc.vector.tensor_mul(
                        out=m_h[:, ko, :], in0=abs_[0], in1=kp_in[0][:, ko, :]
                    )
                    for s in (1, 2):
                        t = tmppool.tile([P, r], BF16, name="t", tag="tmp")
                        nc.vector.tensor_mul(out=t, in0=abs_[s], in1=kp_in[s][:, ko, :])
                        nc.vector.tensor_add(
                            out=m_h[:, ko, :], in0=m_h[:, ko, :], in1=t
                        )
                m_tiles.append(m_h)

            def emit_kproj(s):
                """K projection + score products for kv position s (tile i)."""
                for jo in range(KO):
                    kp = psq.tile([P, r], F32, name="kp", tag="mm")
                    for ko in range(KO):
                        _mm(nc, kp, w_sb["wkT"][:, ko, jo, :], k_in[s][:, ko, :],
                            ko == 0, ko == KO - 1)
                    kc = kcpool.tile([P, r], BF16, name="kc", tag="kc")
                    nc.scalar.activation(
                        out=kc, in_=kp, func=AF.Identity,
                        bias=bias_sb["bk"][:, jo : jo + 1],
                    )
                    pr = prodpool.tile([P, r], BF16, name="pr", tag="pr")
                    nc.vector.tensor_mul(out=pr, in0=qc[:, jo, :], in1=kc)
                    prods[(s, jo)] = pr

            def emit_sel(s, last_s):
                for jo in range(KO):
                    _mm(nc, scores_ps, selscore_sb[:, s * H + jo // 2, :],
                        prods.pop((s, jo)),
                        s == 0 and jo == 0, last_s and jo == KO - 1)

            # interleave: ab/mix groups (tile i-1) between K-proj blocks
            # (tile i) so the PE never waits on the DVE/GpSimd mixing, and
            # sel reductions trail their K block so the products are ready.
            if prev and cur:
                emit_kproj(0)
                emit_ab_mix(0)
                emit_sel(0, False)
                emit_kproj(1)
                emit_ab_mix(1)
                emit_sel(1, False)
                emit_kproj(2)
                emit_ab_mix(2)
                emit_sel(2, True)
            elif cur:
                for s in range(S):
                    emit_kproj(s)
                for s in range(S):
                    emit_sel(s, s == S - 1)
            elif prev:
                for h in range(H):
                    emit_ab_mix(h)

            if cur:
                exp_sb = smpool.tile([SH, r], F32R, name="exp_sb", tag="exp")
                nc.scalar.activation(
                    out=exp_sb, in_=scores_ps, func=AF.Exp, scale=1.0 / 16.0
                )
                state[("exp", i)] = exp_sb

            # ---- tile i-1: V projection of mixed keys, O projection,
            # residual + LayerNorm, store ----
            if prev:
                ctx_sb = ctxpool.tile([P, KO, r], BF16, name="ctx_sb", tag="ctx")
                for h in range(H):
                    for b in range(2):
                        jo = 2 * h + b
                        vp = psq.tile([P, r], F32, name="vp", tag="mm")
                        for ko in range(KO):
                            _mm(nc, vp, w_sb["wvT"][:, ko, jo, :],
                                m_tiles[h][:, ko, :], ko == 0, ko == KO - 1)
                        nc.scalar.activation(
                            out=ctx_sb[:, jo, :], in_=vp, func=AF.Identity,
                            bias=bias_sb["bv"][:, jo : jo + 1],
                        )

                qp_in = state.pop(("q", ip))
                x_sb = xpool.tile([P, KO, r], F32R, name="x_sb", tag="x")
                sx_ps = pssc.tile([1, r], F32, name="sx_ps", tag="sc")
                sxx_ps = pssc.tile([1, r], F32, name="sxx_ps", tag="sc")
                xsqs = []

                def emit_sum(jo):
                    _mm(nc, sx_ps, ones128[:], x_sb[:, jo, :], jo == 0, jo == KO - 1)
                    _mm(nc, sxx_ps, ones128[:], xsqs[jo][:], jo == 0, jo == KO - 1)

                for jo in range(KO):
                    op_ps = psq.tile([P, r], F32, name="op_ps", tag="mm")
                    for ko in range(KO):
                        _mm(nc, op_ps, w_sb["woT"][:, ko, jo, :], ctx_sb[:, ko, :],
                            ko == 0, ko == KO - 1)
                    nc.vector.scalar_tensor_tensor(
                        out=x_sb[:, jo, :], in0=op_ps,
                        scalar=bias_sb["bo"][:, jo : jo + 1],
                        in1=qp_in[:, jo, :], op0=OP.add, op1=OP.add,
                    )
                    xsq = xsqpool.tile([P, r], F32R, name="xsq", tag="xsq")
                    nc.scalar.activation(out=xsq, in_=x_sb[:, jo, :], func=AF.Square)
                    xsqs.append(xsq)
                    if jo >= 1:
                        emit_sum(jo - 1)
                emit_sum(KO - 1)

                mv0 = smpool.tile([1, r], F32, name="mv0", tag="mv0")
                nc.vector.tensor_scalar_mul(out=mv0, in0=sx_ps, scalar1=1.0 / DIM)
                mv1 = smpool.tile([1, r], F32, name="mv1", tag="mv1")
                nc.vector.tensor_scalar_mul(out=mv1, in0=sxx_ps, scalar1=1.0 / DIM)
                mu2 = smpool.tile([1, r], F32, name="mu2", tag="mu2")
                nc.vector.tensor_mul(out=mu2, in0=mv0, in1=mv0)
                # mv1 <- var + eps
                nc.vector.scalar_tensor_tensor(
                    out=mv1, in0=mv1, scalar=EPS, in1=mu2, op0=OP.add, op1=OP.subtract
                )
                rvar = smpool.tile([1, r], F32, name="rvar", tag="rvar")
                nc.vector.reciprocal(out=rvar, in_=mv1)
                rstd = smpool.tile([1, r], F32R, name="rstd", tag="rstd")
                nc.scalar.activation(out=rstd, in_=rvar, func=AF.Sqrt)
                m2 = smpool.tile([1, r], F32R, name="m2", tag="m2")
                nc.vector.tensor_mul(out=m2, in0=mv0, in1=rstd)

                rstd_b = psbc.tile([P, r], F32, name="rstd_b", tag="bc")
                _mm(nc, rstd_b, ones1[:], rstd[:], True, True)
                m2_b = psbc.tile([P, r], F32, name="m2_b", tag="bc")
                _mm(nc, m2_b, ones1[:], m2[:], True, True)
                for jo in range(KO):
                    t1 = t1pool.tile([P, r], F32R, name="t1", tag="t1")
                    nc.vector.tensor_mul(out=t1, in0=x_sb[:, jo, :], in1=rstd_b)
                    nc.vector.tensor_sub(out=t1, in0=t1, in1=m2_b)
                    y = ypool.tile([P, r], F32, name="y", tag="y")
                    nc.scalar.activation(
                        out=y, in_=t1, func=AF.Identity,
                        bias=beta_col[:, jo : jo + 1],
                        scale=gamma_col[:, jo : jo + 1],
                    )
                    nc.gpsimd.dma_start(out=out_t[:, jo, bass.ds(ip * r, r)], in_=y)

        load_tile(0)
        for i in range(niter + 1):
            emit_step(i)

    nc.compile()
    return nc


def make_consts():
    selscore = np.broadcast_to(np.eye(SH, dtype=BF16NP)[None], (P, SH, SH))
    selscore = np.ascontiguousarray(selscore)
    rowsel9 = np.ascontiguousarray(
        np.broadcast_to(np.eye(SH, dtype=BF16NP)[:, :, None], (SH, SH, P))
    )
    k = np.arange(SH)
    sel3 = (k[:, None] % H == np.arange(H)[None, :]).astype(np.float32)
    sel9 = (k[None, :] % H == np.arange(H)[:, None]).astype(np.float32)
    return selscore, rowsel9, sel3, sel9


def make_in_maps(inputs, b_core=BCORE, ncores=NCORES):
    f = np.float32
    q = np.asarray(inputs["query"], f)
    keys = [np.asarray(inputs[f"key{s}"], f) for s in range(S)]
    shared = {
        "wqT": np.ascontiguousarray(np.asarray(inputs["Wq"], f).T).astype(BF16NP),
        "wkT": np.ascontiguousarray(np.asarray(inputs["Wk"], f).T).astype(BF16NP),
        "wvT": np.ascontiguousarray(np.asarray(inputs["Wv"], f).T).astype(BF16NP),
        "woT": np.ascontiguousarray(np.asarray(inputs["Wo"], f).T).astype(BF16NP),
        "bq": np.asarray(inputs["bq"], f),
        "bk": np.asarray(inputs["bk"], f),
        "bv": np.asarray(inputs["bv"], f),
        "bo": np.asarray(inputs["bo"], f),
        "gamma": np.asarray(inputs["gamma"], f),
        "beta": np.asarray(inputs["beta"], f),
    }
    selscore, rowsel9, sel3, sel9 = make_consts()
    shared.update({"selscore": selscore, "rowsel9": rowsel9, "sel3": sel3,
                   "sel9": sel9,
                   "ones128": np.ones((P, 1), f), "ones1": np.ones((1, P), f)})
    in_maps = []
    for c in range(ncores):
        sl = slice(c * b_core, (c + 1) * b_core)
        m = dict(shared)
        m["qT"] = np.ascontiguousarray(q[sl].T).astype(BF16NP)
        for s in range(S):
            m[f"k{s}T"] = np.ascontiguousarray(keys[s][sl].T).astype(BF16NP)
        in_maps.append(m)
    return in_maps


_NC_CACHE = {}


def _get_nc(b_core=BCORE, r=R, niter_cap=None):
    key = (b_core, r, niter_cap)
    if key not in _NC_CACHE:
        _NC_CACHE[key] = build_nc(b_core, r, niter_cap)
    return _NC_CACHE[key]


def run(inputs, trace=False, tmpdir=None):
    """Run on 8 NeuronCores; returns (full output, BassKernelResults)."""
    nc = _get_nc()
    in_maps = make_in_maps(inputs)
    res = run_bass_kernel_spmd(
        nc, in_maps, core_ids=list(range(NCORES)), trace=trace, tmpdir=tmpdir
    )
    y = np.empty((B, DIM), np.float32)
    for c in range(NCORES):
        y[c * BCORE : (c + 1) * BCORE] = res.results[c]["out"].T
    return y, res


def kernel(**inputs):
    y, _ = run(inputs)
    return y
